# revision 17
# baseline (speedup 1.0000x reference)
"""BEVLifter kernel for Trainium2 (8 NeuronCores).

Sharding: 8 cores = 4 samples x 2 image halves (left u<50 / right u>=50).
For a pinhole K, sign(x3) == sign(u - cx), so the left half's scatter
contributions land exactly in BEV columns vx in [0,20) and the right half's
in [20,40): each core owns a disjoint BEV column range end-to-end
(scatter + both BEV convs), no cross-core traffic. The 4-column seam of the
final output (global vx 18..21), whose conv receptive field crosses the
split, is recomputed on the host from raw BEV boundary strips each core
ships out (packed contiguously on-device so the DMA is a few large packets).

Device pipeline (per core, all matmuls bf16 / fp32 PSUM):
  1x1 reduce conv (M=128: weight columns duplicated so PSUM rows 64:128
  hold a copy that a second activation evicts directly into the +1-column
  shifted layout dp1's kx-pair matmuls need - no cross-partition DMA)
  -> 3x3 conv as 3 pair (K=128) + 3 single matmuls + BN + ReLU
  -> 1x1 depth head -> transposed (pixel-major) softmax -> scatter as a
  dense matmul: S[n, d*20+vx] = exp[n,d] * onehot(bx[d,n]==vx), built per
  128-pixel chunk by GPSIMD local_scatter (host-precomputed int16 slot
  indices) or DVE multiply against a host-precomputed one-hot mask;
  P[c, d*20+vx] accumulated in PSUM with ctxT duplicated to M=128 so the
  placement can also fill the x-shifted rows 64:128 of the BEV tile
  -> two 3x3 BEV convs, each 3 pair + 3 single matmuls + BN + ReLU.

A host-side numpy fallback handles inputs whose geometry violates the
structural assumptions (bz varying per pixel, non-sign-splittable bx, ...).
"""

import os

import numpy as np

import concourse.bass as bass
import concourse.bacc as bacc
import concourse.mybir as mybir
import concourse.tile as tile
from concourse.bass_utils import run_bass_kernel_spmd
from concourse.masks import make_identity

try:
    from ml_dtypes import bfloat16 as np_bf16
except Exception:  # pragma: no cover
    import jax.numpy as _jnp
    np_bf16 = _jnp.bfloat16

F32 = np.float32

B, IN_CH, HF, WF = 4, 256, 40, 100
C, D = 64, 48
X0, X1, Z0, Z1, RES = -10.0, 10.0, 0.5, 50.0, 0.5
NX = int((X1 - X0) / RES)  # 40
NZ = int((Z1 - Z0) / RES)  # 99
DMIN, DMAX = 0.5, 50.0
EPS = 1e-5

# per-core tiling
NUC = 50          # image columns owned per core
NU = NUC + 2      # with 1-col halo each side
NV = HF           # 40
NPIXD = NU * NV   # 2080 dense ctx pixels (incl halo cols)
NPIX = NUC * NV   # 2000 core pixels
NCH = 16          # pixel chunks of 128
VP = NV + 2       # padded column height 42
CTXP_W = NU * VP  # 2184
HXW = 20          # bev columns owned per core
SL = D * HXW      # 960 scatter slots
BW = HXW + 2      # padded bev width 22
BH = NZ + 2       # padded bev height 101
BEV_W = BH * BW + 64  # 2286 (pad tail so run-view rearranges stay in range)
OUT_W = NZ * HXW  # 1980
N_GS = 14         # chunks scattered via gpsimd local_scatter (rest: DVE mask)

# blob weight-region offsets (columns, relative to OFF_W)
OFF_W = 2 * NPIXD            # feats region: [0, OFF_W)
_W_WRED = 256                # wred lhsT, 2 K-halves, M duplicated to 128
_W_P9A, _W_S9A = 192, 192    # dp1 pair (128 rows) / single (64 rows)
_W_P9B, _W_S9B = 384, 384    # be1 pair / single, M duplicated to 128
_W_P9C, _W_S9C = 192, 192    # be2 pair / single, M=64
_W_DP2 = 48                  # wdp2 rows 0:64 + b_dp2 row 64
OFF_P9A = OFF_W + _W_WRED
OFF_S9A = OFF_P9A + _W_P9A
OFF_P9B = OFF_S9A + _W_S9A
OFF_S9B = OFF_P9B + _W_P9B
OFF_P9C = OFF_S9B + _W_S9B
OFF_S9C = OFF_P9C + _W_P9C
OFF_WDP2 = OFF_S9C + _W_S9C
OFF_MASK = OFF_WDP2 + _W_DP2

_CACHE = {}


# ---------------------------------------------------------------- device ---
def _build_nc(runs, n_gs):
    dt = mybir.dt
    nc = bacc.Bacc("TRN2", target_bir_lowering=False)
    n_dve = NCH - n_gs
    blob_w = OFF_MASK + max(n_dve, 1) * SL
    d_blob = nc.dram_tensor("blob", [128, blob_w], dt.bfloat16, kind="ExternalInput")
    d_bias = nc.dram_tensor("bias", [128, 8], dt.float32, kind="ExternalInput")
    if n_gs:
        d_lsidx = nc.dram_tensor("lsidx", [128, n_gs * D], dt.int16, kind="ExternalInput")
    d_out = nc.dram_tensor("out", [64, OUT_W], dt.bfloat16, kind="ExternalOutput")
    d_strip = nc.dram_tensor("strip", [64, 8 * NZ], dt.bfloat16, kind="ExternalOutput")

    AF = mybir.ActivationFunctionType
    ALU = mybir.AluOpType
    with (
        tile.TileContext(nc) as tc,
        tc.tile_pool(name="const", bufs=1) as cp,
        tc.tile_pool(name="work", bufs=1) as wp,
        tc.tile_pool(name="stile", bufs=1) as sp,
        tc.tile_pool(name="psum", bufs=3, space="PSUM") as pp,
        tc.tile_pool(name="ptp", bufs=3, space="PSUM") as pt,
        tc.tile_pool(name="pacc", bufs=1, space="PSUM") as pa,
    ):
        blob = cp.tile([128, blob_w], dt.bfloat16)
        bias_st = cp.tile([128, 8], dt.float32)
        biases = cp.tile([128, 8], dt.float32)
        negone = cp.tile([128, 1], dt.float32)
        identD = cp.tile([64, 128], dt.bfloat16)

        # startup DMAs, 3 queues with the critical prefix first on each:
        #   scalar q: biases + wred/dp1/head weights, then be-conv weights
        #   sync q:   feats chunks 0-2, then lsidx + masks (needed ~20us in)
        #   gpsimd q: feats chunks 3-4
        # keeps concurrent streams few so the first matmul's inputs land fast.
        W_CRIT = OFF_S9A + _W_S9A
        nc.scalar.dma_start(out=bias_st[:, :], in_=d_bias[:, :])
        nc.scalar.dma_start(out=blob[:, OFF_W:W_CRIT], in_=d_blob[:, OFF_W:W_CRIT])
        nc.scalar.dma_start(out=blob[:, W_CRIT:OFF_MASK], in_=d_blob[:, W_CRIT:OFF_MASK])
        for a, b in ((0, 800), (800, 1600), (1600, 2400)):
            nc.sync.dma_start(out=blob[:, a:b], in_=d_blob[:, a:b])
        if n_gs:
            lsidx_st = cp.tile([128, n_gs * D], dt.int16)
            lsidx = cp.tile([128, n_gs * D], dt.int16)
            nc.sync.dma_start(out=lsidx_st[:, :], in_=d_lsidx[:, :])
        for i in range(max(n_dve, 1)):
            a, b = OFF_MASK + i * SL, OFF_MASK + (i + 1) * SL
            nc.sync.dma_start(out=blob[:, a:b], in_=d_blob[:, a:b])
        for a, b in ((2400, 3200), (3200, OFF_W)):
            nc.gpsimd.dma_start(out=blob[:, a:b], in_=d_blob[:, a:b])
        nc.vector.tensor_copy(biases[:, :], bias_st[:, :])
        make_identity(nc, identD[:, 0:64])
        make_identity(nc, identD[:, 64:128])

        ctxd = wp.tile([64, 40 + NCH * 128], dt.bfloat16)   # dense ctx, core px at 40+
        ctxp = wp.tile([128, CTXP_W], dt.bfloat16)          # padded ctx; rows 64:128
        #                                                     hold the +1-column shift
        h_sb = wp.tile([65, NCH * 128], dt.bfloat16)
        expT = wp.tile([128, NCH * D], dt.bfloat16)
        ctxT = wp.tile([128, NCH * 128], dt.bfloat16)       # cols duplicated (M=128)
        sumT = wp.tile([128, NCH], dt.float32)
        recipT = wp.tile([128, NCH], dt.float32)
        bev = wp.tile([128, BEV_W], dt.bfloat16)            # rows 64:128: +1-x shift
        bev1 = wp.tile([128, BEV_W], dt.bfloat16)
        outsb = wp.tile([64, OUT_W], dt.bfloat16)
        stag = wp.tile([64, 8 * NZ], dt.bfloat16)           # contiguous strip staging

        # only the regions not later overwritten need zeroing
        nc.vector.memset(ctxd[:, NPIXD:], 0)            # transpose tail pad
        nc.vector.memset(h_sb[0:64, NPIX:], 0)          # softmax tail pad
        nc.vector.memset(h_sb[64:65, :], 1.0)           # bias row for the depth head
        ctxp_v = ctxp[:, :].rearrange("p (u v) -> p u v", u=NU)
        nc.vector.memset(ctxp_v[:, :, 0:1], 0)          # v=0 pad stripe
        nc.vector.memset(ctxp_v[:, :, 41:42], 0)        # v=41 pad stripe
        nc.vector.memset(negone[:, :], -1.0)

        b_red = biases[0:64, 0:1]
        b_red_h = biases[64:128, 0:1]
        b_dp1 = biases[0:64, 1:2]
        b_be1 = biases[0:64, 3:4]
        b_be1_h = biases[64:128, 3:4]
        b_be2 = biases[0:64, 4:5]

        ctxp3 = ctxp[0:64, :].rearrange("p (u v) -> p u v", u=NU)
        ctxp3s = ctxp[64:128, :].rearrange("p (u v) -> p u v", u=NU)
        ctxpF = ctxp[:, :].rearrange("p (u v) -> p u v", u=NU)
        cchunks = [(i * 400, 400) for i in range(4)] + [(1600, 480)]

        # ---- 1x1 reduce conv + BN + ReLU. M=128 (dup weights): rows 0:64
        # evict into the padded layout via scalar, rows 64:128 into the
        # +1-col-shifted rows via a gpsimd relu (add-bias, max-0), keeping
        # the scalar engine off the critical path; dense copy for the
        # transposes on DVE.
        def emit_conv(s0, ln):
            ps = pp.tile([128, 480], dt.float32, tag="convps", name=f"cv{s0}")
            nc.tensor.matmul(out=ps[:, 0:ln], lhsT=blob[:, OFF_W:OFF_W + 128],
                             rhs=blob[:, 2 * s0:2 * s0 + ln], start=True, stop=False)
            nc.tensor.matmul(out=ps[:, 0:ln], lhsT=blob[:, OFF_W + 128:OFF_W + 256],
                             rhs=blob[:, 2 * s0 + ln:2 * s0 + 2 * ln], start=False,
                             stop=True)
            u0, un = s0 // 40, ln // 40
            pslo = ps[0:64, 0:ln].rearrange("p (u v) -> p u v", u=un)
            pshi = ps[64:128, 0:ln].rearrange("p (u v) -> p u v", u=un)
            nc.scalar.activation(out=ctxp3[:, u0:u0 + un, 1:41], in_=pslo,
                                 func=AF.Relu, bias=b_red)
            if u0 == 0:
                nc.vector.tensor_scalar(out=ctxp3s[:, 0:un - 1, 1:41],
                                        in0=pshi[:, 1:un], scalar1=b_red_h,
                                        scalar2=0.0, op0=ALU.add, op1=ALU.max)
            else:
                nc.vector.tensor_scalar(out=ctxp3s[:, u0 - 1:u0 + un - 1, 1:41],
                                        in0=pshi, scalar1=b_red_h,
                                        scalar2=0.0, op0=ALU.add, op1=ALU.max)
            nc.vector.tensor_copy(
                ctxd[:, u0 * 40:(u0 + un) * 40].rearrange("p (u v) -> p u v", u=un),
                ctxp3[:, u0:u0 + un, 1:41])

        # ---- 3x3 conv + BN + ReLU (5 chunks of 10 cols = 400 px) ----
        # kw 0,1 fused per kh via the shifted rows (K=128); kw=2 single (K=64)
        def emit_dp(ci):
            ps = pp.tile([64, 400], dt.float32, tag="convps", name=f"dp{ci}")
            for kh in range(3):
                rhs = ctxpF[:, ci * 10: ci * 10 + 10, kh:kh + 40]
                nc.tensor.matmul(out=ps[:, :].rearrange("p (u v) -> p u v", u=10),
                                 lhsT=blob[:, OFF_P9A + kh * 64:OFF_P9A + (kh + 1) * 64],
                                 rhs=rhs, start=(kh == 0), stop=False)
            for kh in range(3):
                rhs = ctxp3[:, ci * 10 + 2: ci * 10 + 12, kh:kh + 40]
                nc.tensor.matmul(out=ps[:, :].rearrange("p (u v) -> p u v", u=10),
                                 lhsT=blob[0:64, OFF_S9A + kh * 64:OFF_S9A + (kh + 1) * 64],
                                 rhs=rhs, start=False, stop=(kh == 2))
            nc.scalar.activation(out=h_sb[0:64, ci * 400:(ci + 1) * 400], in_=ps[:, :],
                                 func=AF.Relu, bias=b_dp1)

        # ---- per-chunk: transposed softmax + S-build. Softmax uses a constant
        # -1 shift instead of a max-subtract (exact for softmax, logits O(1)).
        # The 1/sum normalization rides the ctxT copy (tensor_scalar_mul), so
        # the S tiles hold raw exp and their build does not wait on recip.
        s_tiles = []

        def emit_head(q):
            lq = slice(q * 128, (q + 1) * 128)
            ltp = pt.tile([128, 48], dt.float32, tag="tp", name=f"lt{q}")
            nc.tensor.matmul(out=ltp[:, :], lhsT=h_sb[0:65, lq],
                             rhs=blob[0:65, OFF_WDP2:OFF_WDP2 + 48],
                             start=True, stop=True)
            nc.scalar.activation(out=expT[:, q * D:(q + 1) * D], in_=ltp[:, :],
                                 func=AF.Exp, bias=negone[:, 0:1],
                                 accum_out=sumT[:, q:q + 1])
            nc.vector.reciprocal(out=recipT[:, q:q + 1], in_=sumT[:, q:q + 1])
            ctp = pt.tile([128, 128], dt.bfloat16, tag="tp", name=f"ct{q}")
            nc.tensor.transpose(out=ctp[:, :], in_=ctxd[:, 40 + q * 128:40 + (q + 1) * 128],
                                identity=identD[0:64, 0:128])
            nc.vector.tensor_scalar_mul(ctxT[:, q * 128:(q + 1) * 128], ctp[:, :],
                                        recipT[:, q:q + 1])

            S = sp.tile([128, SL], dt.bfloat16, tag=f"stile{q}", name=f"s{q}")
            if q < n_gs:
                nc.gpsimd.local_scatter(
                    S[:, :], expT[:, q * D:(q + 1) * D], lsidx[:, q * D:(q + 1) * D],
                    channels=128, num_elems=SL, num_idxs=D)
            else:
                m3 = blob[:, OFF_MASK + (q - n_gs) * SL:
                          OFF_MASK + (q - n_gs + 1) * SL].rearrange(
                    "p (d x) -> p d x", d=D)
                e3 = expT[:, q * D:(q + 1) * D].to_broadcast([128, D, HXW])
                nc.vector.tensor_tensor(out=S[:, :].rearrange("p (d x) -> p d x", d=D),
                                        in0=m3, in1=e3, op=ALU.mult)
            s_tiles.append(S)

        # interleave: conv chunks feed dp1 chunks feed head/S chunks, so the
        # PE never drains (p-state stays high) and the S producers (gpsimd +
        # DVE) start as early as their pixels exist. The big bev memsets ride
        # the engines' idle windows: bev on DVE under the convs, bev1 on
        # gpsimd between its shifted-evict and local_scatter work.
        head_sched = [range(0, 3), range(3, 6), range(6, 9), range(9, 12),
                      range(12, 16)]
        emit_conv(*cchunks[0])
        emit_conv(*cchunks[1])
        nc.vector.memset(bev[:, :], 0)
        if n_gs:
            nc.vector.tensor_copy(lsidx[:, :], lsidx_st[:, :])
        for i in range(5):
            if i + 2 <= 4:
                emit_conv(*cchunks[i + 2])
            emit_dp(i)
            if i == 2:
                nc.gpsimd.memset(bev1[:, :], 0)
            for q in head_sched[i]:
                emit_head(q)

        # ---- scatter matmuls: P[c, d*20+vx], ctxT duplicated so rows 64:128
        # of P are a copy the placement can shift by one x column.
        P = pa.tile([128, SL], dt.float32)
        for q in range(NCH):
            S = s_tiles[q]
            nc.tensor.matmul(out=P[:, 0:512], lhsT=ctxT[:, q * 128:(q + 1) * 128],
                             rhs=S[:, 0:512], start=(q == 0), stop=(q == NCH - 1),
                             skip_group_check=True)
            nc.tensor.matmul(out=P[:, 512:960], lhsT=ctxT[:, q * 128:(q + 1) * 128],
                             rhs=S[:, 512:960], start=(q == 0), stop=(q == NCH - 1),
                             skip_group_check=True)

        # ---- placement: P depth-rows -> bev z-rows (rows 0:64 at x, rows
        # 64:128 at x-1 so the BEV pair matmuls see the +1-x shift) ----
        P3 = P[:, :].rearrange("p (d x) -> p d x", d=D)
        for (d0, ln, bz0, delta) in runs:
            base = (1 + bz0) * BW + 1
            step = delta * BW
            dst = bev[0:64, base:base + ln * step].rearrange("p (g r) -> p g r", r=step)
            nc.vector.tensor_copy(dst[:, :, 0:HXW], P3[0:64, d0:d0 + ln, :])
            dsth = bev[64:128, base - 1:base - 1 + ln * step].rearrange(
                "p (g r) -> p g r", r=step)
            nc.scalar.activation(out=dsth[:, :, 0:HXW], in_=P3[64:128, d0:d0 + ln, :],
                                 func=AF.Copy)

        bev3 = bev[0:64, 0:BH * BW].rearrange("p (z x) -> p z x", z=BH)
        bevF = bev[:, 0:BH * BW].rearrange("p (z x) -> p z x", z=BH)
        bev13 = bev1[0:64, 0:BH * BW].rearrange("p (z x) -> p z x", z=BH)
        bev13h = bev1[64:128, 0:BH * BW].rearrange("p (z x) -> p z x", z=BH)
        bev1F = bev1[:, 0:BH * BW].rearrange("p (z x) -> p z x", z=BH)

        # raw-bev seam strips, packed contiguous then one DMA (the strided
        # 8-byte-element direct DMA was a 40us packet storm)
        stag3 = stag[:, :].rearrange("p (z s) -> p z s", s=8)
        nc.vector.tensor_copy(stag3[:, :, 0:4], bev3[:, 1:1 + NZ, 1:5])
        nc.vector.tensor_copy(stag3[:, :, 4:8], bev3[:, 1:1 + NZ, 17:21])
        nc.gpsimd.dma_start(out=d_strip[:, :], in_=stag[:, :])

        # ---- two 3x3 BEV convs + BN + ReLU, kx 0,1 fused per kz (K=128).
        # conv1 runs on a 21-row z grid and conv2 on a 20-row grid, so each
        # conv2 chunk's 3x3 window only needs conv1 chunks emitted >=2 slots
        # earlier - the PE never waits on an activation that just finished.
        zch1 = [(0, 21), (21, 21), (42, 21), (63, 21), (84, 15)]
        zch2 = [(0, 20), (20, 20), (40, 20), (60, 20), (80, 19)]

        def emit_b1(k):
            z0, nz = zch1[k]
            ps = pp.tile([128, 420], dt.float32, tag="convps", name=f"b1{k}")
            psv = ps[:, 0:nz * HXW].rearrange("p (z x) -> p z x", z=nz)
            for kz in range(3):
                nc.tensor.matmul(out=psv,
                                 lhsT=blob[:, OFF_P9B + kz * 128:OFF_P9B + (kz + 1) * 128],
                                 rhs=bevF[:, z0 + kz:z0 + kz + nz, 0:HXW],
                                 start=(kz == 0), stop=False)
            for kz in range(3):
                nc.tensor.matmul(out=psv,
                                 lhsT=blob[0:64, OFF_S9B + kz * 128:OFF_S9B + (kz + 1) * 128],
                                 rhs=bev3[:, z0 + kz:z0 + kz + nz, 2:2 + HXW],
                                 start=False, stop=(kz == 2))
            pslo = ps[0:64, 0:nz * HXW].rearrange("p (z x) -> p z x", z=nz)
            pshi = ps[64:128, 0:nz * HXW].rearrange("p (z x) -> p z x", z=nz)
            nc.scalar.activation(out=bev13[:, 1 + z0:1 + z0 + nz, 1:1 + HXW],
                                 in_=pslo, func=AF.Relu, bias=b_be1)
            nc.vector.tensor_scalar(out=bev13h[:, 1 + z0:1 + z0 + nz, 0:HXW],
                                    in0=pshi, scalar1=b_be1_h,
                                    scalar2=0.0, op0=ALU.add, op1=ALU.max)

        def emit_b2(k):
            z0, nz = zch2[k]
            ps = pp.tile([64, 400], dt.float32, tag="convps", name=f"b2{k}")
            psv = ps[:, 0:nz * HXW].rearrange("p (z x) -> p z x", z=nz)
            for kz in range(3):
                nc.tensor.matmul(out=psv,
                                 lhsT=blob[:, OFF_P9C + kz * 64:OFF_P9C + (kz + 1) * 64],
                                 rhs=bev1F[:, z0 + kz:z0 + kz + nz, 0:HXW],
                                 start=(kz == 0), stop=False)
            for kz in range(3):
                nc.tensor.matmul(out=psv,
                                 lhsT=blob[0:64, OFF_S9C + kz * 64:OFF_S9C + (kz + 1) * 64],
                                 rhs=bev13[:, z0 + kz:z0 + kz + nz, 2:2 + HXW],
                                 start=False, stop=(kz == 2))
            nc.scalar.activation(
                out=outsb[:, z0 * HXW:(z0 + nz) * HXW].rearrange("p (z x) -> p z x", z=nz),
                in_=psv, func=AF.Relu, bias=b_be2)
            nc.sync.dma_start(out=d_out[:, z0 * HXW:(z0 + nz) * HXW],
                              in_=outsb[:, z0 * HXW:(z0 + nz) * HXW])

        emit_b1(0)
        emit_b1(1)
        emit_b2(0)
        emit_b1(2)
        emit_b2(1)
        emit_b1(3)
        emit_b2(2)
        emit_b1(4)
        emit_b2(3)
        emit_b2(4)

    nc.compile()
    return nc


# ------------------------------------------------------------------ host ---
def _bn_fold(w, b, g, beta, m, v):
    s = (np.asarray(g, F32) / np.sqrt(np.asarray(v, F32) + EPS))
    wf = np.asarray(w, F32) * s.reshape(-1, *([1] * (w.ndim - 1)))
    bf = (np.asarray(b, F32) - np.asarray(m, F32)) * s + np.asarray(beta, F32)
    return wf.astype(F32), bf.astype(F32)


def _pack_pair(w, dup=False):
    # (O, I, 3, 3) -> pair lhsT (128, 3*m): kx=0 rows 0:64 / kx=1 rows 64:128,
    # and single lhsT (64, 3*m): kx=2 -- both indexed by the row-shift k.
    # dup=True duplicates the 64 output columns to M=128 per slab.
    m = 128 if dup else 64
    p = np.zeros((128, 3 * m), F32)
    s = np.zeros((64, 3 * m), F32)
    for k in range(3):
        p[0:64, k * m:k * m + 64] = w[:, :, k, 0].T
        p[64:128, k * m:k * m + 64] = w[:, :, k, 1].T
        s[:, k * m:k * m + 64] = w[:, :, k, 2].T
        if dup:
            p[:, k * m + 64:(k + 1) * m] = p[:, k * m:k * m + 64]
            s[:, k * m + 64:(k + 1) * m] = s[:, k * m:k * m + 64]
    return p, s


def _geometry(K, Hs, Ws):
    """Per-sample geometry; returns dict or None if fast-path assumptions fail."""
    Nf = HF * WF
    scale = np.array([WF / Ws, HF / Hs, 1.0], F32)
    K_s = (np.asarray(K, F32) * scale[None, :, None]).astype(F32)
    vv, uu = np.meshgrid(np.arange(HF, dtype=F32), np.arange(WF, dtype=F32), indexing="ij")
    pix = np.stack([uu, vv, np.ones_like(uu)], 0).reshape(3, Nf)
    dc = np.linspace(DMIN, DMAX, D).astype(F32).reshape(D, 1)
    geos = []
    bzd_ref = None
    for b in range(K_s.shape[0]):
        try:
            Kinv = np.linalg.inv(K_s[b].astype(np.float64)).astype(F32)
        except np.linalg.LinAlgError:
            return None
        rays = (Kinv @ pix).astype(F32)
        x3 = rays[0:1] * dc
        z3 = rays[2:3] * dc
        bx = ((x3 - X0) / RES).astype(np.int32)
        bz = ((z3 - Z0) / RES).astype(np.int32)
        valid = (bx >= 0) & (bx < NX) & (bz >= 0) & (bz < NZ)
        # structural checks for the device fast path
        if not np.all(bz == bz[:, :1]):
            return None
        bzd = bz[:, 0]
        uu_i = (np.arange(Nf) % WF)
        side = (uu_i >= NUC)[None, :]
        if not np.all(~valid | ((bx >= 20) == side)):
            return None
        vb = bzd[(bzd >= 0) & (bzd < NZ)]
        if len(np.unique(vb)) != len(vb):
            return None
        geos.append((bx, valid, bzd))
        if bzd_ref is None:
            bzd_ref = bzd
        elif not np.array_equal(bzd_ref, bzd):
            return None  # placement runs are baked into the (shared) program
    # placement runs: consecutive valid d with constant bz delta
    runs = []
    dvalid = [d for d in range(D) if 0 <= bzd_ref[d] < NZ]
    i = 0
    while i < len(dvalid):
        d0 = dvalid[i]
        j = i
        delta = None
        while j + 1 < len(dvalid) and dvalid[j + 1] == dvalid[j] + 1:
            step = int(bzd_ref[dvalid[j + 1]] - bzd_ref[dvalid[j]])
            if step <= 0 or (1 + bzd_ref[d0]) * BW + 1 + (j + 1 - i) * step * BW + HXW > BEV_W:
                break
            if delta is None:
                delta = step
            elif step != delta:
                break
            j += 1
        runs.append((d0, j - i + 1, int(bzd_ref[d0]), delta or 1))
        i = j + 1
    return {"geos": geos, "runs": tuple(runs)}


def _core_tables(geo, side):
    """masks (128, n_dve*SL) bf16 and lsidx (128, n_gs*D) int16 for one core."""
    bx, valid, bzd = geo
    u0 = side * NUC
    p = np.arange(NCH * 128)
    u_loc, v = p // NV, p % NV
    n = v * WF + (u0 + u_loc)
    ok = p < NPIX
    n = np.where(ok, n, 0)
    bxp = bx[:, n]                      # (D, 2048)
    vp = valid[:, n] & ok[None, :]
    zok = ((bzd >= 0) & (bzd < NZ))[:, None]
    x = bxp - 20 * side
    hit = vp & zok & (x >= 0) & (x < HXW)
    n_dve = NCH - N_GS
    masks = np.zeros((128, max(n_dve, 1) * SL), np_bf16)
    lsidx = np.full((128, max(N_GS, 1) * D), -1, np.int16)
    slot = np.arange(D)[:, None] * HXW + np.clip(x, 0, HXW - 1)  # (D, 2048)
    for q in range(NCH):
        part = slice(0, 128)
        pq = slice(q * 128, (q + 1) * 128)
        if q < N_GS:
            idx = np.where(hit[:, pq], slot[:, pq], -1).T.astype(np.int16)  # (128, D)
            lsidx[part, q * D:(q + 1) * D] = idx
        else:
            mq = np.zeros((128, D, HXW), F32)
            hh = hit[:, pq].T                    # (128, D)
            xx = np.clip(x[:, pq].T, 0, HXW - 1)  # (128, D)
            pi, di = np.nonzero(hh)
            mq[pi, di, xx[pi, di]] = 1.0
            masks[part, (q - N_GS) * SL:(q - N_GS + 1) * SL] = \
                mq.reshape(128, SL).astype(np_bf16)
    return masks, lsidx


def _magic_col(wred_f, bred_f):
    # feats column f0 with relu(W f0 + b) == 0 (pre-act forced to -1)
    W = wred_f.astype(np.float64)
    rhs = -(bred_f.astype(np.float64) + 1.0)
    f0 = W.T @ np.linalg.solve(W @ W.T, rhs)
    return f0.astype(F32)


def _host_reference(encoder_features, K, H, W, weights):
    """Pure-numpy fallback, exact port of the reference."""
    (wred_f, bred_f, wdp1_f, bdp1_f, wdp2_f, bdp2_f,
     wbe1_f, bbe1_f, wbe2_f, bbe2_f) = weights
    feats = np.asarray(encoder_features, F32)
    Nf = HF * WF

    def conv3x3(x, w, b):
        Bb, Ci, Hh, Ww = x.shape
        xp = np.zeros((Bb, Ci, Hh + 2, Ww + 2), F32)
        xp[:, :, 1:-1, 1:-1] = x
        y = np.zeros((Bb, w.shape[0], Hh, Ww), F32)
        for ky in range(3):
            for kx in range(3):
                y += np.einsum("oi,bihw->bohw", w[:, :, ky, kx],
                               xp[:, :, ky:ky + Hh, kx:kx + Ww], optimize=True)
        return y + b[None, :, None, None]

    ctx = np.maximum(np.einsum("oi,bihw->bohw", wred_f[:, :, 0, 0], feats, optimize=True)
                     + bred_f[None, :, None, None], 0.0)
    h = np.maximum(conv3x3(ctx, wdp1_f, bdp1_f), 0.0)
    logits = np.einsum("oi,bihw->bohw", wdp2_f[:, :, 0, 0], h, optimize=True) \
        + bdp2_f[None, :, None, None]
    lm = logits.max(axis=1, keepdims=True)
    e = np.exp(logits - lm)
    prob = e / e.sum(axis=1, keepdims=True)

    scale = np.array([WF / W, HF / H, 1.0], F32)
    K_s = np.asarray(K, F32) * scale[None, :, None]
    vv, uu = np.meshgrid(np.arange(HF, dtype=F32), np.arange(WF, dtype=F32), indexing="ij")
    pix = np.stack([uu, vv, np.ones_like(uu)], 0).reshape(3, Nf)
    dc = np.linspace(DMIN, DMAX, D).astype(F32).reshape(1, D, 1)
    bev = np.zeros((B, C, NZ * NX), F32)
    ctxf = ctx.reshape(B, C, Nf)
    probf = prob.reshape(B, D, Nf)
    nidx = np.tile(np.arange(Nf, dtype=np.int64)[None, :], (D, 1)).ravel()
    for b in range(B):
        rays = np.linalg.inv(K_s[b].astype(np.float64)).astype(F32) @ pix
        x3 = rays[0:1] * dc[0]
        z3 = rays[2:3] * dc[0]
        bxb = ((x3 - X0) / RES).astype(np.int32)
        bzb = ((z3 - Z0) / RES).astype(np.int32)
        validb = (bxb >= 0) & (bxb < NX) & (bzb >= 0) & (bzb < NZ)
        idxb = np.clip(bzb * NX + bxb, 0, NZ * NX - 1).reshape(D * Nf)
        Mn = np.zeros((Nf, NZ * NX), F32)
        wfl = (probf[b] * validb).ravel()
        np.add.at(Mn, (nidx, idxb), wfl)
        bev[b] = ctxf[b] @ Mn
    bev = bev.reshape(B, C, NZ, NX)
    bev = np.maximum(conv3x3(bev, wbe1_f, bbe1_f), 0.0)
    bev = np.maximum(conv3x3(bev, wbe2_f, bbe2_f), 0.0)
    return bev.astype(F32)


def kernel(encoder_features, K, H, W,
           w_red, b_red, g_red, be_red, m_red, v_red,
           w_dp1, b_dp1, g_dp1, be_dp1, m_dp1, v_dp1,
           w_dp2, b_dp2,
           w_be1, b_be1, g_be1, be_be1, m_be1, v_be1,
           w_be2, b_be2, g_be2, be_be2, m_be2, v_be2):
    feats = np.asarray(encoder_features, F32)
    K = np.asarray(K, F32)
    Hs = float(np.asarray(H))
    Ws = float(np.asarray(W))

    wred_f, bred_f = _bn_fold(np.asarray(w_red, F32)[:, :, 0, 0], b_red, g_red, be_red,
                              m_red, v_red)
    wdp1_f, bdp1_f = _bn_fold(np.asarray(w_dp1, F32), b_dp1, g_dp1, be_dp1, m_dp1, v_dp1)
    wdp2_f = np.asarray(w_dp2, F32)
    bdp2_f = np.asarray(b_dp2, F32)
    wbe1_f, bbe1_f = _bn_fold(np.asarray(w_be1, F32), b_be1, g_be1, be_be1, m_be1, v_be1)
    wbe2_f, bbe2_f = _bn_fold(np.asarray(w_be2, F32), b_be2, g_be2, be_be2, m_be2, v_be2)
    weights = (wred_f.reshape(64, 256, 1, 1), bred_f, wdp1_f, bdp1_f,
               wdp2_f, bdp2_f, wbe1_f, bbe1_f, wbe2_f, bbe2_f)

    gkey = (K.tobytes(), Hs, Ws)
    if _CACHE.get("gkey") != gkey:
        g = _geometry(K, Hs, Ws)
        _CACHE["gkey"] = gkey
        _CACHE["geom"] = g
        _CACHE.pop("tables", None)
    g = _CACHE["geom"]
    if g is None:
        return _host_reference(feats, K, Hs, Ws, weights)

    if "tables" not in _CACHE:
        _CACHE["tables"] = [
            _core_tables(g["geos"][core // 2], core % 2) for core in range(8)]
    nckey = (g["runs"], N_GS)
    if _CACHE.get("nckey") != nckey:
        _CACHE["nc"] = _build_nc(g["runs"], N_GS)
        _CACHE["nckey"] = nckey
    nc = _CACHE["nc"]

    # per-core inputs (blob layout must match _build_nc)
    n_dve = NCH - N_GS
    blob_w = OFF_MASK + max(n_dve, 1) * SL
    wblob = np.zeros((128, OFF_MASK - OFF_W), np_bf16)
    wredT = wred_f.T.astype(np_bf16)           # (256, 64)
    wblob[:, 0:64] = wredT[0:128]
    wblob[:, 64:128] = wredT[0:128]
    wblob[:, 128:192] = wredT[128:256]
    wblob[:, 192:256] = wredT[128:256]
    o = 256
    p, s = _pack_pair(wdp1_f)
    wblob[:, o:o + 192] = p.astype(np_bf16)
    wblob[0:64, o + 192:o + 384] = s.astype(np_bf16)
    o += 384
    p, s = _pack_pair(wbe1_f, dup=True)
    wblob[:, o:o + 384] = p.astype(np_bf16)
    wblob[0:64, o + 384:o + 768] = s.astype(np_bf16)
    o += 768
    p, s = _pack_pair(wbe2_f)
    wblob[:, o:o + 192] = p.astype(np_bf16)
    wblob[0:64, o + 192:o + 384] = s.astype(np_bf16)
    o += 384
    wblob[0:64, o:o + 48] = np.ascontiguousarray(wdp2_f[:, :, 0, 0].T).astype(np_bf16)
    wblob[64, o:o + 48] = bdp2_f.astype(np_bf16)
    assert OFF_W + o + 48 == OFF_MASK

    biases = np.zeros((128, 8), F32)
    for col, bvec in ((0, bred_f), (1, bdp1_f), (3, bbe1_f), (4, bbe2_f)):
        biases[0:64, col] = bvec
        biases[64:128, col] = bvec
    biases[0:48, 2] = bdp2_f
    f0 = _magic_col(wred_f, bred_f)

    featsT = np.transpose(feats, (0, 1, 3, 2))  # (B, 256, WF, HF)
    in_maps = []
    for core in range(8):
        b, side = core // 2, core % 2
        u0 = side * NUC
        arr = np.zeros((IN_CH, NU, NV), F32)
        ulo, uhi = u0 - 1, u0 + NUC + 1
        slo, shi = max(ulo, 0), min(uhi, WF)
        arr[:, slo - ulo:shi - ulo, :] = featsT[b][:, slo:shi, :]
        if ulo < 0:
            arr[:, 0, :] = f0[:, None]
        if uhi > WF:
            arr[:, NU - 1, :] = f0[:, None]
        fb = arr.reshape(IN_CH, NPIXD).astype(np_bf16)
        masks, lsidx = _CACHE["tables"][core]
        bl = np.zeros((128, blob_w), np_bf16)
        for (s0, ln) in [(i * 400, 400) for i in range(4)] + [(1600, 480)]:
            bl[:, 2 * s0:2 * s0 + ln] = fb[0:128, s0:s0 + ln]
            bl[:, 2 * s0 + ln:2 * s0 + 2 * ln] = fb[128:256, s0:s0 + ln]
        bl[:, OFF_W:OFF_MASK] = wblob
        bl[:, OFF_MASK:] = masks
        im = {"blob": bl, "bias": biases}
        if N_GS:
            im["lsidx"] = lsidx
        in_maps.append(im)

    trace = os.environ.get("BEV_TRACE") == "1"
    run_kwargs = {}
    if trace:
        run_kwargs["trace"] = True
        td = os.environ.get("BEV_TRACE_DIR")
        if td:
            n = _CACHE.get("ncalls", 0)
            _CACHE["ncalls"] = n + 1
            sub = os.path.join(td, f"run{n}")
            os.makedirs(sub, exist_ok=True)
            run_kwargs["tmpdir"] = sub
    res = run_bass_kernel_spmd(nc, in_maps, core_ids=list(range(8)), **run_kwargs)
    _CACHE["exec_time_ns"] = getattr(res, "exec_time_ns", None)
    _CACHE["profile_json"] = getattr(res, "profile_json", None)
    _CACHE["trace_path"] = getattr(res, "instructions_and_trace", None)

    out = np.zeros((B, C, NZ, NX), F32)
    strips = np.zeros((B, C, NZ, 8), F32)  # global vx 16..24
    for core in range(8):
        b, side = core // 2, core % 2
        o = np.asarray(res.results[core]["out"], F32).reshape(C, NZ, HXW)
        out[b, :, :, side * HXW:(side + 1) * HXW] = o
        st = np.asarray(res.results[core]["strip"], F32).reshape(C, NZ, 8)
        if side == 0:
            strips[b, :, :, 0:4] = st[:, :, 4:8]   # local vx 16..20 = global 16..20
        else:
            strips[b, :, :, 4:8] = st[:, :, 0:4]   # local vx 0..4 = global 20..24

    # host fixup of the 4-column seam (global vx 18..22)
    raw = np.zeros((B, C, NZ + 2, 10), F32)
    raw[:, :, 1:-1, 1:9] = strips                     # cols 16..24 at idx 1..9
    y1 = np.zeros((B, C, NZ + 2, 8), F32)             # cols 17..23 at idx 1..7
    for kz in range(3):
        for kx in range(3):
            y1[:, :, 1:-1, 1:7] += np.einsum(
                "oi,bizx->bozx", wbe1_f[:, :, kz, kx],
                raw[:, :, kz:kz + NZ, kx + 1:kx + 7], optimize=True)
    y1[:, :, 1:-1, 1:7] += bbe1_f[None, :, None, None]
    y1 = np.maximum(y1, 0.0)
    y1[:, :, 0] = 0.0
    y1[:, :, -1] = 0.0
    y2 = np.zeros((B, C, NZ, 4), F32)                 # cols 18..22
    for kz in range(3):
        for kx in range(3):
            y2 += np.einsum("oi,bizx->bozx", wbe2_f[:, :, kz, kx],
                            y1[:, :, kz:kz + NZ, kx + 1:kx + 5], optimize=True)
    y2 = np.maximum(y2 + bbe2_f[None, :, None, None], 0.0)
    out[:, :, :, 18:22] = y2
    return out.astype(F32)
